# revision 46
# baseline (speedup 1.0000x reference)
"""SuperposedExpert, token-parallel variant: 8-way DP, no collectives.

Each core handles 256 tokens and runs ALL 4 paths over them. TT cores are
expanded to dense W1/W2 on the host (with (1 + path_weight) folded into W2).

v6 structure: merged windows. Window W_0 is path-0 ffn1; window W_k
(k=1..3) interleaves ffn2 of path k-1 (8 matmuls/iter, f=512 h-stationary,
output in [tok, d] layout) with ffn1 of path k (16 matmuls/iter, f=256,
4-chain blocks), 16 iters per window. This stretches every weight stream's
deadline (w1[k+1] group g is needed at +6.9*2g us into the window instead
of +6.9*g) so the HBM ramp in path 0/1 stops stalling the PE, and the PE
p-state never drops. PSUM: 4 banks for the ffn1 block chains (pp1), 4 for
the ffn2 window chains + gating (pp2). Gating is transposed (logits
[tok, K] via x-stationary matmuls, softmax on the free axis), so gates are
per-partition scalars fused into the ffn2 drains (ps2 * gexp * rden).
The last path's ffn2 runs chain-outer with w2[3] fully resident (second
half parked in freed w1pool slots), draining each chain + output DMA under
the next chain's matmuls.
"""

import numpy as np
import ml_dtypes

import concourse.bass as bass
import concourse.tile as tile
from concourse import bacc, mybir
from concourse.bass import ds, ts
from concourse.bass_utils import run_bass_kernel_spmd

BF16 = mybir.dt.bfloat16
F32 = mybir.dt.float32
AF = mybir.ActivationFunctionType
ALU = mybir.AluOpType

K = 4
D = 1024
DFF = 4096
NTOK = 2048
NCORES = 8
NTC = NTOK // NCORES   # 256 tokens per core


def _emit(nc, tc):
    xTp = nc.dram_tensor("xTp", [128, 8, NTC], BF16, kind="ExternalInput")
    # W1 packed f-quarter-major: [g][p][s][fq] so each quarter is one
    # contiguous-HBM 2MB DMA and ffn1 group g starts after (g+1) quarters
    w1p = [nc.dram_tensor(f"w1p{k}", [4, 128, 8, DFF // 4], BF16,
                          kind="ExternalInput") for k in range(K)]
    w2p = [nc.dram_tensor(f"w2p{k}", [128, 32, D], BF16, kind="ExternalInput")
           for k in range(K)]
    pbT = nc.dram_tensor("pbT", [D, K], BF16, kind="ExternalInput")
    opiece = nc.dram_tensor("opiece", [128, 2, D], BF16, kind="ExternalOutput")

    with (
        tc.tile_pool(name="w1pool", bufs=4) as w1pool,
        # 8 bufs of 1MB: tile (k, Q) recycles (k-1, Q)'s slot, which dies at
        # ffn2-(k-1) iter 2Q+1 right when the trigger fires — no ring
        # stalls, and the last path holds all 8 slices for chain-outer
        tc.tile_pool(name="w2pool", bufs=8) as w2pool,
        tc.tile_pool(name="htp", bufs=2) as htp,
        tc.tile_pool(name="small", bufs=1) as small,
        tc.tile_pool(name="obp", bufs=4) as obp,
        # pp1: ffn1 block chains (full-bank tiles: an accumulation start
        # zeroes a whole 2KB zero-region); pp2: ffn2 window chains + gating
        tc.tile_pool(name="pp1", bufs=4, space="PSUM") as pp1,
        tc.tile_pool(name="pp2", bufs=4, space="PSUM") as pp2,
    ):
        # x s-slice 0 lands first (64KB) so the first ffn1 matmul can fire
        # as soon as the first W1 slice is in
        xt_sb = small.tile([128, 8, NTC], BF16, tag="xt")
        pbt_sb = small.tile([128, 8, K], BF16, tag="pbt")
        nc.sync.dma_start(xt_sb[:, 0], xTp[:, 0, :])

        w1q = [[w1pool.tile([128, 8, DFF // 4], BF16, tag="w1",
                            name=f"w1_{k}_{g}") for g in range(4)]
               for k in range(K)]
        # group 0 streams in s-slices (region-level hazards let the s=0
        # matmuls start after the first 256KB instead of a whole 2MB
        # quarter); later groups as progressively coarser DMAs, all on the
        # sync ring which is the only one pulling at t=0. pbt (needed only
        # by the gating at the end of group 0) queues behind the hot path.
        for lo, n in ((0, 1), (1, 2), (3, 2), (5, 3)):
            nc.sync.dma_start(w1q[0][0][:, ds(lo, n)],
                              w1p[0][0][:, ds(lo, n), :])
            if lo == 0:
                nc.sync.dma_start(xt_sb[:, ds(1, 7)], xTp[:, ds(1, 7), :])
            elif lo == 1:
                nc.sync.dma_start(
                    pbt_sb, pbT.ap().rearrange("(t p) k -> p t k", p=128))
        for h in range(2):
            nc.sync.dma_start(w1q[0][1][:, ds(4 * h, 4)],
                              w1p[0][1][:, ds(4 * h, 4), :])
        for g in range(2, 4):
            nc.sync.dma_start(w1q[0][g], w1p[0][g])

        # gating state: gexp[tok, th, k] = exp(logit), rden[tok, th] = 1/sum
        gexp = small.tile([128, 2, K], F32, tag="gexp")
        rden = small.tile([128, 2], F32, tag="rden")
        den = small.tile([128, 2], F32, tag="den")
        acc = small.tile([128, 2, D], F32, tag="acc")

        ht_t = [None] * K
        w2tls = [[None] * 8 for _ in range(K)]

        def w2_load(k, q, eng):
            w2tls[k][q] = w2pool.tile([128, 4, D], BF16, tag="w2",
                                      name=f"w2_{k}_{q}")
            eng.dma_start(w2tls[k][q], w2p[k][:, ds(4 * q, 4), :])

        def emit_gating():
            # logits^T [tok, K] via x-stationary matmuls (f=4, trivial PE
            # time), softmax along the free axis.
            for th in range(2):
                lg = pp2.tile([128, K], F32, tag="ps2", name=f"lg{th}")
                for s in range(8):
                    nc.tensor.matmul(lg, xt_sb[:, s, ts(th, 128)],
                                     pbt_sb[:, s],
                                     start=(s == 0), stop=(s == 7))
                nc.scalar.activation(gexp[:, th], lg, AF.Exp)
            for th in range(2):
                nc.vector.tensor_reduce(den[:, ds(th, 1)], gexp[:, th],
                                        mybir.AxisListType.X, ALU.add)
                nc.vector.reciprocal(rden[:, ds(th, 1)], den[:, ds(th, 1)])

        def ffn1_half(k, b, half, ps1):
            # 16 matmuls: chains j=0..3 of block b=(g, jh), s-steps
            # 4*half..4*half+3; each chain owns a full PSUM bank.
            g, jh = b // 2, b % 2
            for s in range(4 * half, 4 * half + 4):
                for j in range(4):
                    jj = jh * 4 + j
                    nc.tensor.matmul(ps1[j][:, ds(0, NTC)],
                                     w1q[k][g][:, s, ts(jj, 128)],
                                     xt_sb[:, s],
                                     start=(s == 0), stop=(s == 7))
            if half == 1:
                cols = g * 8 + jh * 4
                for j in range(4):
                    nc.scalar.activation(ht_t[k][:, cols + j],
                                         ps1[j][:, ds(0, NTC)],
                                         AF.Gelu_apprx_tanh)

        def new_ps1(k, b):
            return [pp1.tile([128, 512], F32, tag="ps1",
                             name=f"f1_{k}_{b}_{j}") for j in range(4)]

        def ffn2_step(k, ps2, q):
            for kc in range(2):
                s2 = 2 * q + kc
                for th in range(2):
                    for dc in range(2):
                        nc.tensor.matmul(
                            ps2[th * 2 + dc], ht_t[k][:, s2, ts(th, 128)],
                            w2tls[k][s2 // 4][:, s2 % 4, ts(dc, 512)],
                            start=(s2 == 0), stop=(s2 == 31))

        def drain(k, ps2c, th, dc):
            g1 = gexp[:, th, ds(k, 1)]
            g2 = rden[:, ds(th, 1)]
            c = th * 2 + dc
            if k == 0:
                nc.vector.tensor_scalar(
                    acc[:, th, ts(dc, 512)], ps2c, g1, g2,
                    ALU.mult, ALU.mult)
            elif k < K - 1:
                ob = obp.tile([128, 512], F32, tag="ob", name=f"ob_{k}_{c}")
                nc.vector.tensor_scalar(ob, ps2c, g1, g2, ALU.mult, ALU.mult)
                nc.vector.tensor_add(acc[:, th, ts(dc, 512)],
                                     acc[:, th, ts(dc, 512)], ob)
            else:
                ob = obp.tile([128, 512], F32, tag="ob", name=f"ob_{k}_{c}")
                nc.vector.tensor_scalar(ob, ps2c, g1, g2, ALU.mult, ALU.mult)
                obf = obp.tile([128, 512], BF16, tag="obf", name=f"obf_{c}")
                nc.vector.tensor_add(obf, acc[:, th, ts(dc, 512)], ob)
                nc.sync.dma_start(opiece[:, th, ts(dc, 512)], obf)

        # ---- W_0: path-0 ffn1 standalone + gating ----
        # 4-chain blocks alternating pp1 (combined tile, one 4-wide gelu)
        # and pp2 (singles): consecutive blocks use different pools, so a
        # block never waits on the previous block's gelu.
        ht_t[0] = htp.tile([128, 32, NTC], BF16, tag="ht", name="ht_0")
        # 8-chain groups (no concurrent ffn2, so borrow pp2's 4 banks):
        # gelu of group g overlaps group g+1's chains with no WAR stall.
        # Triggers fire in delivery-deadline order per ring, and w2 tiles
        # allocate in Q-ascending order so pool slots line up with death
        # order (tile (k,Q) always recycles (k-1,Q)'s slot). Sync takes
        # Q0-1 behind the W1 stream; scalar takes w1[1] + Q2-3.
        sched = [[("w2s", 0), ("w1", 0)],
                 [("w2s", 1), ("w2", 2), ("w1", 1)],
                 [("w2", 3), ("w1", 2)],
                 [("w1", 3)]]
        for grp in range(4):
            ps1 = [(pp1 if j < 4 else pp2).tile(
                [128, 512], F32, tag=("ps1" if j < 4 else "ps2"),
                name=f"f1_0_{grp}_{j}") for j in range(8)]
            for s in range(8):
                for j in range(8):
                    nc.tensor.matmul(ps1[j][:, ds(0, NTC)],
                                     w1q[0][grp][:, s, ts(j, 128)],
                                     xt_sb[:, s],
                                     start=(s == 0), stop=(s == 7))
            for j in range(8):
                nc.scalar.activation(ht_t[0][:, grp * 8 + j],
                                     ps1[j][:, ds(0, NTC)],
                                     AF.Gelu_apprx_tanh)
            if grp == 0:
                emit_gating()
            for kind, idx in sched[grp]:
                if kind == "w1":
                    nc.scalar.dma_start(w1q[1][idx], w1p[1][idx])
                elif kind == "w2s":
                    w2_load(0, idx, nc.sync)
                else:
                    w2_load(0, idx, nc.scalar)

        # ---- W_k (k=1..3): ffn2 of k-1 interleaved with ffn1 of k ----
        for k in range(1, K):
            prev = k - 1
            ht_t[k] = htp.tile([128, 32, NTC], BF16, tag="ht", name=f"ht_{k}")
            ps2 = [pp2.tile([128, 512], F32, tag="ps2", name=f"f2_{prev}_{c}")
                   for c in range(4)]
            ps1 = None
            for q in range(16):
                ffn2_step(prev, ps2, q)
                b, half = q // 2, q % 2
                if half == 0:
                    ps1 = new_ps1(k, b)
                ffn1_half(k, b, half, ps1)
                # triggers (block ends / early iters), in ring-FIFO
                # need-order; all WAR-free by the time they're reached
                if q < 4:
                    w2_load(prev, q + 4, nc.scalar)
                if half == 1:
                    if b < 4:
                        w2_load(k, b, nc.scalar)
                    elif k == K - 1:
                        # last path: no ffn2-k3 window follows, so pull its
                        # w2 second half in as (2, Q4..7) slots free up
                        w2_load(k, b, nc.scalar)
                    if q % 4 == 3 and k + 1 < K:
                        g = q // 4
                        nc.scalar.dma_start(w1q[k + 1][g], w1p[k + 1][g])
            for th in range(2):
                for dc in range(2):
                    drain(prev, ps2[th * 2 + dc], th, dc)

        # ---- ffn2 of the last path: chain-outer, w2 fully resident ----
        for th in range(2):
            for dc in range(2):
                ps2c = pp2.tile([128, 512], F32, tag="ps2",
                                name=f"f2_3_{th}{dc}")
                for s2 in range(32):
                    nc.tensor.matmul(
                        ps2c, ht_t[3][:, s2, ts(th, 128)],
                        w2tls[3][s2 // 4][:, s2 % 4, ts(dc, 512)],
                        start=(s2 == 0), stop=(s2 == 31))
                drain(3, ps2c, th, dc)


def build(verbose=False):
    nc = bacc.Bacc("TRN2", target_bir_lowering=False, debug=False, num_devices=NCORES)
    with tile.TileContext(nc) as tc:
        _emit(nc, tc)
    nc.compile()
    return nc


def _expand_tt(core1, core2, din, dout):
    a, x, r = core1.shape
    r2, b, y = core2.shape
    m = core1.reshape(a * x, r).astype(np.float32) @ \
        core2.reshape(r2, b * y).astype(np.float32)
    w = m.reshape(a, x, b, y).transpose(0, 2, 1, 3).reshape(a * b, x * y)
    assert w.shape == (din, dout)
    return w


def make_in_maps(inputs):
    tokens = inputs["tokens"]
    bf = ml_dtypes.bfloat16
    shared = {}
    for k in range(K):
        w1 = _expand_tt(inputs["ffn1_core1"][k], inputs["ffn1_core2"][k], D, DFF)
        # [(s p), (g fq)] -> [g, p, s, fq]
        shared[f"w1p{k}"] = np.ascontiguousarray(
            w1.reshape(8, 128, 4, DFF // 4).transpose(2, 1, 0, 3)).astype(bf)
        w2 = _expand_tt(inputs["ffn2_core1"][k], inputs["ffn2_core2"][k], DFF, D)
        w2 *= (1.0 + inputs["path_weights"][k])[None, :]
        shared[f"w2p{k}"] = np.ascontiguousarray(
            w2.reshape(32, 128, D).transpose(1, 0, 2)).astype(bf)
    shared["pbT"] = np.ascontiguousarray(inputs["path_bases"].T).astype(bf)
    in_maps = []
    for c in range(NCORES):
        tok = tokens[c * NTC:(c + 1) * NTC]
        xt = np.ascontiguousarray(
            tok.T.reshape(8, 128, NTC).transpose(1, 0, 2)).astype(bf)
        m = dict(shared)
        m["xTp"] = xt
        in_maps.append(m)
    return in_maps


def assemble(results):
    out = np.empty((NTOK, D), np.float32)
    for c in range(NCORES):
        # piece [128 p, 2 th, 1024 d]; token = c*256 + th*128 + p
        piece = results[c]["opiece"].astype(np.float32)
        out[c * NTC:(c + 1) * NTC] = piece.transpose(1, 0, 2).reshape(NTC, D)
    return out


_NC = None


def run(inputs, trace=False):
    global _NC
    if _NC is None:
        _NC = build()
    res = run_bass_kernel_spmd(
        _NC, make_in_maps(inputs), core_ids=list(range(NCORES)), trace=trace
    )
    return assemble(res.results), res


def kernel(**inputs):
    out, _ = run(inputs)
    return out


# revision 53
# speedup vs baseline: 1.0133x; 1.0133x over previous
"""SuperposedExpert, token-parallel variant: 8-way DP, no collectives.

Each core handles 256 tokens and runs ALL 4 paths over them. TT cores are
expanded to dense W1/W2 on the host (with (1 + path_weight) folded into W2).

v6 structure: merged windows. Window W_0 is path-0 ffn1; window W_k
(k=1..3) interleaves ffn2 of path k-1 (8 matmuls/iter, f=512 h-stationary,
output in [tok, d] layout) with ffn1 of path k (16 matmuls/iter, f=256,
4-chain blocks), 16 iters per window. This stretches every weight stream's
deadline (w1[k+1] group g is needed at +6.9*2g us into the window instead
of +6.9*g) so the HBM ramp in path 0/1 stops stalling the PE, and the PE
p-state never drops. PSUM: 4 banks for the ffn1 block chains (pp1), 4 for
the ffn2 window chains + gating (pp2). Gating is transposed (logits
[tok, K] via x-stationary matmuls, softmax on the free axis), so gates are
per-partition scalars fused into the ffn2 drains (ps2 * gexp * rden).
The last path's ffn2 runs chain-outer with w2[3] fully resident (second
half parked in freed w1pool slots), draining each chain + output DMA under
the next chain's matmuls.
"""

import numpy as np
import ml_dtypes

import concourse.bass as bass
import concourse.tile as tile
from concourse import bacc, mybir
from concourse.bass import ds, ts
from concourse.bass_utils import run_bass_kernel_spmd

BF16 = mybir.dt.bfloat16
F32 = mybir.dt.float32
AF = mybir.ActivationFunctionType
ALU = mybir.AluOpType

K = 4
D = 1024
DFF = 4096
NTOK = 2048
NCORES = 8
NTC = NTOK // NCORES   # 256 tokens per core


def _emit(nc, tc):
    xTp = nc.dram_tensor("xTp", [128, 8, NTC], BF16, kind="ExternalInput")
    # W1 packed f-quarter-major: [g][p][s][fq] so each quarter is one
    # contiguous-HBM 2MB DMA and ffn1 group g starts after (g+1) quarters
    w1p = [nc.dram_tensor(f"w1p{k}", [4, 128, 8, DFF // 4], BF16,
                          kind="ExternalInput") for k in range(K)]
    w2p = [nc.dram_tensor(f"w2p{k}", [128, 32, D], BF16, kind="ExternalInput")
           for k in range(K)]
    pbT = nc.dram_tensor("pbT", [D, K], BF16, kind="ExternalInput")
    opiece = nc.dram_tensor("opiece", [128, 2, D], BF16, kind="ExternalOutput")

    with (
        tc.tile_pool(name="w1pool", bufs=4) as w1pool,
        # 8 bufs of 1MB: tile (k, Q) recycles (k-1, Q)'s slot, which dies at
        # ffn2-(k-1) iter 2Q+1 right when the trigger fires — no ring
        # stalls, and the last path holds all 8 slices for chain-outer
        tc.tile_pool(name="w2pool", bufs=8) as w2pool,
        tc.tile_pool(name="htp", bufs=2) as htp,
        tc.tile_pool(name="small", bufs=1) as small,
        tc.tile_pool(name="obp", bufs=4) as obp,
        # pp1: ffn1 block chains (full-bank tiles: an accumulation start
        # zeroes a whole 2KB zero-region); pp2: ffn2 window chains + gating
        tc.tile_pool(name="pp1", bufs=4, space="PSUM") as pp1,
        tc.tile_pool(name="pp2", bufs=4, space="PSUM") as pp2,
    ):
        # x s-slice 0 lands first (64KB) so the first ffn1 matmul can fire
        # as soon as the first W1 slice is in
        xt_sb = small.tile([128, 8, NTC], BF16, tag="xt")
        pbt_sb = small.tile([128, 8, K], BF16, tag="pbt")
        nc.sync.dma_start(xt_sb[:, 0], xTp[:, 0, :])

        w1q = [[w1pool.tile([128, 8, DFF // 4], BF16, tag="w1",
                            name=f"w1_{k}_{g}") for g in range(4)]
               for k in range(K)]
        # group 0 streams in s-slices (region-level hazards let the s=0
        # matmuls start after the first 256KB instead of a whole 2MB
        # quarter); later groups as progressively coarser DMAs, all on the
        # sync ring which is the only one pulling at t=0. pbt (needed only
        # by the gating at the end of group 0) queues behind the hot path.
        for lo, n in ((0, 1), (1, 2), (3, 2), (5, 3)):
            nc.sync.dma_start(w1q[0][0][:, ds(lo, n)],
                              w1p[0][0][:, ds(lo, n), :])
            if lo == 0:
                nc.sync.dma_start(xt_sb[:, ds(1, 7)], xTp[:, ds(1, 7), :])
            elif lo == 1:
                nc.sync.dma_start(
                    pbt_sb, pbT.ap().rearrange("(t p) k -> p t k", p=128))
        for h in range(2):
            nc.sync.dma_start(w1q[0][1][:, ds(4 * h, 4)],
                              w1p[0][1][:, ds(4 * h, 4), :])
        for g in range(2, 4):
            nc.sync.dma_start(w1q[0][g], w1p[0][g])

        # gating state: gexp[tok, th, k] = exp(logit), rden[tok, th] = 1/sum
        gexp = small.tile([128, 2, K], F32, tag="gexp")
        rden = small.tile([128, 2], F32, tag="rden")
        den = small.tile([128, 2], F32, tag="den")
        acc = small.tile([128, 2, D], F32, tag="acc")

        ht_t = [None] * K
        w2tls = [[None] * 8 for _ in range(K)]

        def w2_load(k, q, eng):
            w2tls[k][q] = w2pool.tile([128, 4, D], BF16, tag="w2",
                                      name=f"w2_{k}_{q}")
            eng.dma_start(w2tls[k][q], w2p[k][:, ds(4 * q, 4), :])

        def emit_gating():
            # logits^T [tok, K] via x-stationary matmuls (f=4, trivial PE
            # time), softmax along the free axis.
            for th in range(2):
                lg = pp2.tile([128, K], F32, tag="ps2", name=f"lg{th}")
                for s in range(8):
                    nc.tensor.matmul(lg, xt_sb[:, s, ts(th, 128)],
                                     pbt_sb[:, s],
                                     start=(s == 0), stop=(s == 7))
                nc.scalar.activation(gexp[:, th], lg, AF.Exp)
            for th in range(2):
                nc.vector.tensor_reduce(den[:, ds(th, 1)], gexp[:, th],
                                        mybir.AxisListType.X, ALU.add)
                nc.vector.reciprocal(rden[:, ds(th, 1)], den[:, ds(th, 1)])

        def ffn1_half(k, b, half, ps1):
            # 16 matmuls: chains j=0..3 of block b=(g, jh), s-steps
            # 4*half..4*half+3; each chain owns a full PSUM bank.
            g, jh = b // 2, b % 2
            for s in range(4 * half, 4 * half + 4):
                for j in range(4):
                    jj = jh * 4 + j
                    nc.tensor.matmul(ps1[j][:, ds(0, NTC)],
                                     w1q[k][g][:, s, ts(jj, 128)],
                                     xt_sb[:, s],
                                     start=(s == 0), stop=(s == 7))
            if half == 1:
                cols = g * 8 + jh * 4
                for j in range(4):
                    nc.scalar.activation(ht_t[k][:, cols + j],
                                         ps1[j][:, ds(0, NTC)],
                                         AF.Gelu_apprx_tanh)

        def new_ps1(k, b):
            return [pp1.tile([128, 512], F32, tag="ps1",
                             name=f"f1_{k}_{b}_{j}") for j in range(4)]

        def ffn2_step(k, ps2, q):
            for kc in range(2):
                s2 = 2 * q + kc
                for th in range(2):
                    for dc in range(2):
                        nc.tensor.matmul(
                            ps2[th * 2 + dc], ht_t[k][:, s2, ts(th, 128)],
                            w2tls[k][s2 // 4][:, s2 % 4, ts(dc, 512)],
                            start=(s2 == 0), stop=(s2 == 31))

        def drain(k, ps2c, th, dc):
            g1 = gexp[:, th, ds(k, 1)]
            g2 = rden[:, ds(th, 1)]
            c = th * 2 + dc
            if k == 0:
                nc.vector.tensor_scalar(
                    acc[:, th, ts(dc, 512)], ps2c, g1, g2,
                    ALU.mult, ALU.mult)
            elif k < K - 1:
                ob = obp.tile([128, 512], F32, tag="ob", name=f"ob_{k}_{c}")
                nc.vector.tensor_scalar(ob, ps2c, g1, g2, ALU.mult, ALU.mult)
                nc.vector.tensor_add(acc[:, th, ts(dc, 512)],
                                     acc[:, th, ts(dc, 512)], ob)
            else:
                ob = obp.tile([128, 512], F32, tag="ob", name=f"ob_{k}_{c}")
                nc.vector.tensor_scalar(ob, ps2c, g1, g2, ALU.mult, ALU.mult)
                obf = obp.tile([128, 512], BF16, tag="obf", name=f"obf_{c}")
                nc.vector.tensor_add(obf, acc[:, th, ts(dc, 512)], ob)
                nc.sync.dma_start(opiece[:, th, ts(dc, 512)], obf)

        # ---- W_0: path-0 ffn1 standalone + gating ----
        # 4-chain blocks alternating pp1 (combined tile, one 4-wide gelu)
        # and pp2 (singles): consecutive blocks use different pools, so a
        # block never waits on the previous block's gelu.
        ht_t[0] = htp.tile([128, 32, NTC], BF16, tag="ht", name="ht_0")
        # 8-chain groups (no concurrent ffn2, so borrow pp2's 4 banks):
        # gelu of group g overlaps group g+1's chains with no WAR stall.
        # Triggers fire in delivery-deadline order per ring, and w2 tiles
        # allocate in Q-ascending order so pool slots line up with death
        # order (tile (k,Q) always recycles (k-1,Q)'s slot). Sync takes
        # Q0-1 behind the W1 stream; scalar takes w1[1] + Q2-3.
        sched = [[("w2s", 0), ("w1", 0)],
                 [("w2s", 1), ("w2", 2), ("w1", 1)],
                 [("w2", 3), ("w1", 2)],
                 [("w1", 3)]]
        for grp in range(4):
            ps1 = [(pp1 if j < 4 else pp2).tile(
                [128, 512], F32, tag=("ps1" if j < 4 else "ps2"),
                name=f"f1_0_{grp}_{j}") for j in range(8)]
            for s in range(8):
                for j in range(8):
                    nc.tensor.matmul(ps1[j][:, ds(0, NTC)],
                                     w1q[0][grp][:, s, ts(j, 128)],
                                     xt_sb[:, s],
                                     start=(s == 0), stop=(s == 7))
            for j in range(8):
                nc.scalar.activation(ht_t[0][:, grp * 8 + j],
                                     ps1[j][:, ds(0, NTC)],
                                     AF.Gelu_apprx_tanh)
            if grp == 0:
                emit_gating()
            for kind, idx in sched[grp]:
                if kind == "w1":
                    nc.scalar.dma_start(w1q[1][idx], w1p[1][idx])
                elif kind == "w2s":
                    w2_load(0, idx, nc.sync)
                else:
                    w2_load(0, idx, nc.scalar)

        # ---- W_k (k=1..3): ffn2 of k-1 interleaved with ffn1 of k ----
        for k in range(1, K):
            prev = k - 1
            ht_t[k] = htp.tile([128, 32, NTC], BF16, tag="ht", name=f"ht_{k}")
            ps2 = [pp2.tile([128, 512], F32, tag="ps2", name=f"f2_{prev}_{c}")
                   for c in range(4)]
            ps1 = None
            for q in range(16):
                ffn2_step(prev, ps2, q)
                b, half = q // 2, q % 2
                if half == 0:
                    ps1 = new_ps1(k, b)
                ffn1_half(k, b, half, ps1)
                # triggers (block ends / early iters), in ring-FIFO
                # need-order; all WAR-free by the time they're reached
                if q < 4:
                    w2_load(prev, q + 4, nc.scalar)
                if half == 1:
                    if b < 4:
                        w2_load(k, b, nc.scalar)
                    elif k == K - 1:
                        # last path: no ffn2-k3 window follows, so pull its
                        # w2 second half in as (2, Q4..7) slots free up
                        w2_load(k, b, nc.scalar)
                    if q % 4 == 3 and k + 1 < K:
                        g = q // 4
                        nc.scalar.dma_start(w1q[k + 1][g], w1p[k + 1][g])
            for th in range(2):
                for dc in range(2):
                    drain(prev, ps2[th * 2 + dc], th, dc)

        # ---- ffn2 of the last path: chain-outer, w2 fully resident ----
        for th in range(2):
            for dc in range(2):
                ps2c = pp2.tile([128, 512], F32, tag="ps2",
                                name=f"f2_3_{th}{dc}")
                for s2 in range(32):
                    nc.tensor.matmul(
                        ps2c, ht_t[3][:, s2, ts(th, 128)],
                        w2tls[3][s2 // 4][:, s2 % 4, ts(dc, 512)],
                        start=(s2 == 0), stop=(s2 == 31))
                drain(3, ps2c, th, dc)


def build(verbose=False):
    nc = bacc.Bacc("TRN2", target_bir_lowering=False, debug=False, num_devices=NCORES)
    with tile.TileContext(nc) as tc:
        _emit(nc, tc)
    nc.compile()
    return nc


def _expand_tt(core1, core2, din, dout):
    a, x, r = core1.shape
    r2, b, y = core2.shape
    m = core1.reshape(a * x, r).astype(np.float32) @ \
        core2.reshape(r2, b * y).astype(np.float32)
    w = m.reshape(a, x, b, y).transpose(0, 2, 1, 3).reshape(a * b, x * y)
    assert w.shape == (din, dout)
    return w


def make_in_maps(inputs):
    tokens = inputs["tokens"]
    bf = ml_dtypes.bfloat16
    shared = {}
    for k in range(K):
        w1 = _expand_tt(inputs["ffn1_core1"][k], inputs["ffn1_core2"][k], D, DFF)
        # [(s p), (g fq)] -> [g, p, s, fq]
        shared[f"w1p{k}"] = np.ascontiguousarray(
            w1.reshape(8, 128, 4, DFF // 4).transpose(2, 1, 0, 3)).astype(bf)
        w2 = _expand_tt(inputs["ffn2_core1"][k], inputs["ffn2_core2"][k], DFF, D)
        w2 *= (1.0 + inputs["path_weights"][k])[None, :]
        shared[f"w2p{k}"] = np.ascontiguousarray(
            w2.reshape(32, 128, D).transpose(1, 0, 2)).astype(bf)
    shared["pbT"] = np.ascontiguousarray(inputs["path_bases"].T).astype(bf)
    in_maps = []
    for c in range(NCORES):
        tok = tokens[c * NTC:(c + 1) * NTC]
        xt = np.ascontiguousarray(
            tok.T.reshape(8, 128, NTC).transpose(1, 0, 2)).astype(bf)
        m = dict(shared)
        m["xTp"] = xt
        in_maps.append(m)
    return in_maps


def assemble(results):
    out = np.empty((NTOK, D), np.float32)
    for c in range(NCORES):
        # piece [128 p, 2 th, 1024 d]; token = c*256 + th*128 + p
        piece = results[c]["opiece"].astype(np.float32)
        out[c * NTC:(c + 1) * NTC] = piece.transpose(1, 0, 2).reshape(NTC, D)
    return out


_NC = None


def run(inputs, trace=False):
    global _NC
    if _NC is None:
        _NC = build()
    res = run_bass_kernel_spmd(
        _NC, make_in_maps(inputs), core_ids=list(range(NCORES)), trace=trace
    )
    return assemble(res.results), res


def kernel(**inputs):
    out, _ = run(inputs)
    return out


# revision 54
# speedup vs baseline: 1.0398x; 1.0262x over previous
"""SuperposedExpert, token-parallel variant: 8-way DP, no collectives.

Each core handles 256 tokens and runs ALL 4 paths over them. TT cores are
expanded to dense W1/W2 on the host (with (1 + path_weight) folded into W2).

v6 structure: merged windows. Window W_0 is path-0 ffn1; window W_k
(k=1..3) interleaves ffn2 of path k-1 (8 matmuls/iter, f=512 h-stationary,
output in [tok, d] layout) with ffn1 of path k (16 matmuls/iter, f=256,
4-chain blocks), 16 iters per window. This stretches every weight stream's
deadline (w1[k+1] group g is needed at +6.9*2g us into the window instead
of +6.9*g) so the HBM ramp in path 0/1 stops stalling the PE, and the PE
p-state never drops. PSUM: 4 banks for the ffn1 block chains (pp1), 4 for
the ffn2 window chains + gating (pp2). Gating is transposed (logits
[tok, K] via x-stationary matmuls, softmax on the free axis), so gates are
per-partition scalars fused into the ffn2 drains (ps2 * gexp * rden).
The last path's ffn2 runs chain-outer with w2[3] fully resident (second
half parked in freed w1pool slots), draining each chain + output DMA under
the next chain's matmuls.
"""

import numpy as np
import ml_dtypes

import concourse.bass as bass
import concourse.tile as tile
from concourse import bacc, mybir
from concourse.bass import ds, ts
from concourse.bass_utils import run_bass_kernel_spmd

BF16 = mybir.dt.bfloat16
F32 = mybir.dt.float32
AF = mybir.ActivationFunctionType
ALU = mybir.AluOpType

K = 4
D = 1024
DFF = 4096
NTOK = 2048
NCORES = 8
NTC = NTOK // NCORES   # 256 tokens per core


def _emit(nc, tc):
    xTp = nc.dram_tensor("xTp", [128, 8, NTC], BF16, kind="ExternalInput")
    # W1 packed f-quarter-major: [g][p][s][fq] so each quarter is one
    # contiguous-HBM 2MB DMA and ffn1 group g starts after (g+1) quarters
    w1p = [nc.dram_tensor(f"w1p{k}", [4, 128, 8, DFF // 4], BF16,
                          kind="ExternalInput") for k in range(K)]
    w2p = [nc.dram_tensor(f"w2p{k}", [128, 32, D], BF16, kind="ExternalInput")
           for k in range(K)]
    pbT = nc.dram_tensor("pbT", [D, K], BF16, kind="ExternalInput")
    opiece = nc.dram_tensor("opiece", [128, 2, D], BF16, kind="ExternalOutput")

    with (
        tc.tile_pool(name="w1pool", bufs=4) as w1pool,
        # 8 bufs of 1MB: tile (k, Q) recycles (k-1, Q)'s slot, which dies at
        # ffn2-(k-1) iter 2Q+1 right when the trigger fires — no ring
        # stalls, and the last path holds all 8 slices for chain-outer
        tc.tile_pool(name="w2pool", bufs=8) as w2pool,
        tc.tile_pool(name="htp", bufs=2) as htp,
        tc.tile_pool(name="small", bufs=1) as small,
        tc.tile_pool(name="obp", bufs=4) as obp,
        # pp1: ffn1 block chains (full-bank tiles: an accumulation start
        # zeroes a whole 2KB zero-region); pp2: ffn2 window chains + gating
        tc.tile_pool(name="pp1", bufs=4, space="PSUM") as pp1,
        tc.tile_pool(name="pp2", bufs=4, space="PSUM") as pp2,
    ):
        # x s-slice 0 lands first (64KB) so the first ffn1 matmul can fire
        # as soon as the first W1 slice is in
        xt_sb = small.tile([128, 8, NTC], BF16, tag="xt")
        pbt_sb = small.tile([128, 8, K], BF16, tag="pbt")
        nc.sync.dma_start(xt_sb[:, 0], xTp[:, 0, :])

        w1q = [[w1pool.tile([128, 8, DFF // 4], BF16, tag="w1",
                            name=f"w1_{k}_{g}") for g in range(4)]
               for k in range(K)]
        # group 0 streams in s-slices (region-level hazards let the s=0
        # matmuls start after the first 256KB instead of a whole 2MB
        # quarter); later groups as progressively coarser DMAs, all on the
        # sync ring which is the only one pulling at t=0. pbt (needed only
        # by the gating at the end of group 0) queues behind the hot path.
        for lo, n in ((0, 1), (1, 2), (3, 2), (5, 3)):
            nc.sync.dma_start(w1q[0][0][:, ds(lo, n)],
                              w1p[0][0][:, ds(lo, n), :])
            if lo == 0:
                nc.sync.dma_start(xt_sb[:, ds(1, 7)], xTp[:, ds(1, 7), :])
            elif lo == 1:
                nc.sync.dma_start(
                    pbt_sb, pbT.ap().rearrange("(t p) k -> p t k", p=128))
        for h in range(2):
            nc.sync.dma_start(w1q[0][1][:, ds(4 * h, 4)],
                              w1p[0][1][:, ds(4 * h, 4), :])
        for g in range(2, 4):
            nc.sync.dma_start(w1q[0][g], w1p[0][g])

        # gating state: gexp[tok, th, k] = exp(logit), rden[tok, th] = 1/sum
        gexp = small.tile([128, 2, K], F32, tag="gexp")
        rden = small.tile([128, 2], F32, tag="rden")
        den = small.tile([128, 2], F32, tag="den")
        acc = small.tile([128, 2, D], F32, tag="acc")

        ht_t = [None] * K
        w2tls = [[None] * 8 for _ in range(K)]

        def w2_load(k, q, eng):
            w2tls[k][q] = w2pool.tile([128, 4, D], BF16, tag="w2",
                                      name=f"w2_{k}_{q}")
            eng.dma_start(w2tls[k][q], w2p[k][:, ds(4 * q, 4), :])

        def emit_gating():
            # logits^T [tok, K] via x-stationary matmuls (f=4, trivial PE
            # time), softmax along the free axis.
            for th in range(2):
                lg = pp2.tile([128, K], F32, tag="ps2", name=f"lg{th}")
                for s in range(8):
                    nc.tensor.matmul(lg, xt_sb[:, s, ts(th, 128)],
                                     pbt_sb[:, s],
                                     start=(s == 0), stop=(s == 7))
                nc.scalar.activation(gexp[:, th], lg, AF.Exp)
            for th in range(2):
                nc.vector.tensor_reduce(den[:, ds(th, 1)], gexp[:, th],
                                        mybir.AxisListType.X, ALU.add)
                nc.vector.reciprocal(rden[:, ds(th, 1)], den[:, ds(th, 1)])

        def ffn1_half(k, b, half, ps1):
            # 16 matmuls: chains j=0..3 of block b=(g, jh), s-steps
            # 4*half..4*half+3; each chain owns a full PSUM bank.
            g, jh = b // 2, b % 2
            for s in range(4 * half, 4 * half + 4):
                for j in range(4):
                    jj = jh * 4 + j
                    nc.tensor.matmul(ps1[j][:, ds(0, NTC)],
                                     w1q[k][g][:, s, ts(jj, 128)],
                                     xt_sb[:, s],
                                     start=(s == 0), stop=(s == 7))
            if half == 1:
                cols = g * 8 + jh * 4
                for j in range(4):
                    nc.scalar.activation(ht_t[k][:, cols + j],
                                         ps1[j][:, ds(0, NTC)],
                                         AF.Gelu_apprx_tanh)

        def new_ps1(k, b):
            return [pp1.tile([128, 512], F32, tag="ps1",
                             name=f"f1_{k}_{b}_{j}") for j in range(4)]

        def ffn2_step(k, ps2, q):
            for kc in range(2):
                s2 = 2 * q + kc
                for th in range(2):
                    for dc in range(2):
                        nc.tensor.matmul(
                            ps2[th * 2 + dc], ht_t[k][:, s2, ts(th, 128)],
                            w2tls[k][s2 // 4][:, s2 % 4, ts(dc, 512)],
                            start=(s2 == 0), stop=(s2 == 31))

        def drain(k, ps2c, th, dc):
            g1 = gexp[:, th, ds(k, 1)]
            g2 = rden[:, ds(th, 1)]
            c = th * 2 + dc
            if k == 0:
                nc.vector.tensor_scalar(
                    acc[:, th, ts(dc, 512)], ps2c, g1, g2,
                    ALU.mult, ALU.mult)
            elif k < K - 1:
                ob = obp.tile([128, 512], F32, tag="ob", name=f"ob_{k}_{c}")
                nc.vector.tensor_scalar(ob, ps2c, g1, g2, ALU.mult, ALU.mult)
                nc.vector.tensor_add(acc[:, th, ts(dc, 512)],
                                     acc[:, th, ts(dc, 512)], ob)
            else:
                ob = obp.tile([128, 512], F32, tag="ob", name=f"ob_{k}_{c}")
                nc.vector.tensor_scalar(ob, ps2c, g1, g2, ALU.mult, ALU.mult)
                obf = obp.tile([128, 512], BF16, tag="obf", name=f"obf_{c}")
                nc.vector.tensor_add(obf, acc[:, th, ts(dc, 512)], ob)
                nc.sync.dma_start(opiece[:, th, ts(dc, 512)], obf)

        # ---- W_0: path-0 ffn1 standalone + gating ----
        # 4-chain blocks alternating pp1 (combined tile, one 4-wide gelu)
        # and pp2 (singles): consecutive blocks use different pools, so a
        # block never waits on the previous block's gelu.
        ht_t[0] = htp.tile([128, 32, NTC], BF16, tag="ht", name="ht_0")
        # 8-chain groups (no concurrent ffn2, so borrow pp2's 4 banks):
        # gelu of group g overlaps group g+1's chains with no WAR stall.
        # Triggers fire in delivery-deadline order per ring, and w2 tiles
        # allocate in Q-ascending order so pool slots line up with death
        # order (tile (k,Q) always recycles (k-1,Q)'s slot). Sync takes
        # Q0-1 behind the W1 stream; scalar takes w1[1] + Q2-3.
        sched = [[("w2s", 0), ("w1", 0)],
                 [("w2s", 1), ("w2", 2), ("w1", 1)],
                 [("w2", 3), ("w1", 2)],
                 [("w1", 3)]]
        for grp in range(4):
            ps1 = [(pp1 if j < 4 else pp2).tile(
                [128, 512], F32, tag=("ps1" if j < 4 else "ps2"),
                name=f"f1_0_{grp}_{j}") for j in range(8)]
            for s in range(8):
                for j in range(8):
                    nc.tensor.matmul(ps1[j][:, ds(0, NTC)],
                                     w1q[0][grp][:, s, ts(j, 128)],
                                     xt_sb[:, s],
                                     start=(s == 0), stop=(s == 7))
            for j in range(8):
                nc.scalar.activation(ht_t[0][:, grp * 8 + j],
                                     ps1[j][:, ds(0, NTC)],
                                     AF.Gelu_apprx_tanh)
            if grp == 0:
                emit_gating()
            for kind, idx in sched[grp]:
                if kind == "w1":
                    nc.scalar.dma_start(w1q[1][idx], w1p[1][idx])
                elif kind == "w2s":
                    w2_load(0, idx, nc.sync)
                else:
                    w2_load(0, idx, nc.scalar)

        # ---- W_k (k=1..3): ffn2 of k-1 interleaved with ffn1 of k ----
        for k in range(1, K):
            prev = k - 1
            ht_t[k] = htp.tile([128, 32, NTC], BF16, tag="ht", name=f"ht_{k}")
            ps2 = [pp2.tile([128, 512], F32, tag="ps2", name=f"f2_{prev}_{c}")
                   for c in range(4)]
            ps1 = None
            for q in range(16):
                ffn2_step(prev, ps2, q)
                b, half = q // 2, q % 2
                if half == 0:
                    ps1 = new_ps1(k, b)
                ffn1_half(k, b, half, ps1)
                # triggers (block ends / early iters), in ring-FIFO
                # need-order; all WAR-free by the time they're reached.
                # W_1's Q4-7 prefetches ride sync (idle once the W_0 ramp
                # drains) instead of queueing behind 10MB of scalar freight.
                if q < 4:
                    w2_load(prev, q + 4, nc.sync if k == 1 else nc.scalar)
                if half == 1:
                    if b < 4:
                        w2_load(k, b, nc.scalar)
                    elif k == K - 1:
                        # last path: no ffn2-k3 window follows, so pull its
                        # w2 second half in as (2, Q4..7) slots free up
                        w2_load(k, b, nc.scalar)
                    if q % 4 == 3 and k + 1 < K:
                        g = q // 4
                        nc.scalar.dma_start(w1q[k + 1][g], w1p[k + 1][g])
            for th in range(2):
                for dc in range(2):
                    drain(prev, ps2[th * 2 + dc], th, dc)

        # ---- ffn2 of the last path: chain-outer, w2 fully resident ----
        for th in range(2):
            for dc in range(2):
                ps2c = pp2.tile([128, 512], F32, tag="ps2",
                                name=f"f2_3_{th}{dc}")
                for s2 in range(32):
                    nc.tensor.matmul(
                        ps2c, ht_t[3][:, s2, ts(th, 128)],
                        w2tls[3][s2 // 4][:, s2 % 4, ts(dc, 512)],
                        start=(s2 == 0), stop=(s2 == 31))
                drain(3, ps2c, th, dc)


def build(verbose=False):
    nc = bacc.Bacc("TRN2", target_bir_lowering=False, debug=False, num_devices=NCORES)
    with tile.TileContext(nc) as tc:
        _emit(nc, tc)
    nc.compile()
    return nc


def _expand_tt(core1, core2, din, dout):
    a, x, r = core1.shape
    r2, b, y = core2.shape
    m = core1.reshape(a * x, r).astype(np.float32) @ \
        core2.reshape(r2, b * y).astype(np.float32)
    w = m.reshape(a, x, b, y).transpose(0, 2, 1, 3).reshape(a * b, x * y)
    assert w.shape == (din, dout)
    return w


def make_in_maps(inputs):
    tokens = inputs["tokens"]
    bf = ml_dtypes.bfloat16
    shared = {}
    for k in range(K):
        w1 = _expand_tt(inputs["ffn1_core1"][k], inputs["ffn1_core2"][k], D, DFF)
        # [(s p), (g fq)] -> [g, p, s, fq]
        shared[f"w1p{k}"] = np.ascontiguousarray(
            w1.reshape(8, 128, 4, DFF // 4).transpose(2, 1, 0, 3)).astype(bf)
        w2 = _expand_tt(inputs["ffn2_core1"][k], inputs["ffn2_core2"][k], DFF, D)
        w2 *= (1.0 + inputs["path_weights"][k])[None, :]
        shared[f"w2p{k}"] = np.ascontiguousarray(
            w2.reshape(32, 128, D).transpose(1, 0, 2)).astype(bf)
    shared["pbT"] = np.ascontiguousarray(inputs["path_bases"].T).astype(bf)
    in_maps = []
    for c in range(NCORES):
        tok = tokens[c * NTC:(c + 1) * NTC]
        xt = np.ascontiguousarray(
            tok.T.reshape(8, 128, NTC).transpose(1, 0, 2)).astype(bf)
        m = dict(shared)
        m["xTp"] = xt
        in_maps.append(m)
    return in_maps


def assemble(results):
    out = np.empty((NTOK, D), np.float32)
    for c in range(NCORES):
        # piece [128 p, 2 th, 1024 d]; token = c*256 + th*128 + p
        piece = results[c]["opiece"].astype(np.float32)
        out[c * NTC:(c + 1) * NTC] = piece.transpose(1, 0, 2).reshape(NTC, D)
    return out


_NC = None


def run(inputs, trace=False):
    global _NC
    if _NC is None:
        _NC = build()
    res = run_bass_kernel_spmd(
        _NC, make_in_maps(inputs), core_ids=list(range(NCORES)), trace=trace
    )
    return assemble(res.results), res


def kernel(**inputs):
    out, _ = run(inputs)
    return out


# revision 56
# speedup vs baseline: 1.0431x; 1.0032x over previous
"""SuperposedExpert, token-parallel variant: 8-way DP, no collectives.

Each core handles 256 tokens and runs ALL 4 paths over them. TT cores are
expanded to dense W1/W2 on the host (with (1 + path_weight) folded into W2).

v6 structure: merged windows. Window W_0 is path-0 ffn1; window W_k
(k=1..3) interleaves ffn2 of path k-1 (8 matmuls/iter, f=512 h-stationary,
output in [tok, d] layout) with ffn1 of path k (16 matmuls/iter, f=256,
4-chain blocks), 16 iters per window. This stretches every weight stream's
deadline (w1[k+1] group g is needed at +6.9*2g us into the window instead
of +6.9*g) so the HBM ramp in path 0/1 stops stalling the PE, and the PE
p-state never drops. PSUM: 4 banks for the ffn1 block chains (pp1), 4 for
the ffn2 window chains + gating (pp2). Gating is transposed (logits
[tok, K] via x-stationary matmuls, softmax on the free axis), so gates are
per-partition scalars fused into the ffn2 drains (ps2 * gexp * rden).
The last path's ffn2 runs chain-outer with w2[3] fully resident (second
half parked in freed w1pool slots), draining each chain + output DMA under
the next chain's matmuls.
"""

import numpy as np
import ml_dtypes

import concourse.bass as bass
import concourse.tile as tile
from concourse import bacc, mybir
from concourse.bass import ds, ts
from concourse.bass_utils import run_bass_kernel_spmd

BF16 = mybir.dt.bfloat16
F32 = mybir.dt.float32
AF = mybir.ActivationFunctionType
ALU = mybir.AluOpType

K = 4
D = 1024
DFF = 4096
NTOK = 2048
NCORES = 8
NTC = NTOK // NCORES   # 256 tokens per core


def _emit(nc, tc):
    xTp = nc.dram_tensor("xTp", [128, 8, NTC], BF16, kind="ExternalInput")
    # W1 packed f-quarter-major: [g][p][s][fq] so each quarter is one
    # contiguous-HBM 2MB DMA and ffn1 group g starts after (g+1) quarters
    w1p = [nc.dram_tensor(f"w1p{k}", [4, 128, 8, DFF // 4], BF16,
                          kind="ExternalInput") for k in range(K)]
    w2p = [nc.dram_tensor(f"w2p{k}", [128, 32, D], BF16, kind="ExternalInput")
           for k in range(K)]
    pbT = nc.dram_tensor("pbT", [D, K], BF16, kind="ExternalInput")
    opiece = nc.dram_tensor("opiece", [128, 2, D], BF16, kind="ExternalOutput")

    with (
        tc.tile_pool(name="w1pool", bufs=4) as w1pool,
        # 8 bufs of 1MB: tile (k, Q) recycles (k-1, Q)'s slot, which dies at
        # ffn2-(k-1) iter 2Q+1 right when the trigger fires — no ring
        # stalls, and the last path holds all 8 slices for chain-outer
        tc.tile_pool(name="w2pool", bufs=8) as w2pool,
        tc.tile_pool(name="htp", bufs=2) as htp,
        tc.tile_pool(name="small", bufs=1) as small,
        tc.tile_pool(name="obp", bufs=4) as obp,
        # pp1: ffn1 block chains (full-bank tiles: an accumulation start
        # zeroes a whole 2KB zero-region); pp2: ffn2 window chains + gating
        tc.tile_pool(name="pp1", bufs=4, space="PSUM") as pp1,
        tc.tile_pool(name="pp2", bufs=4, space="PSUM") as pp2,
    ):
        # x s-slice 0 lands first (64KB) so the first ffn1 matmul can fire
        # as soon as the first W1 slice is in
        xt_sb = small.tile([128, 8, NTC], BF16, tag="xt")
        pbt_sb = small.tile([128, 8, K], BF16, tag="pbt")
        nc.sync.dma_start(xt_sb[:, 0], xTp[:, 0, :])

        w1q = [[w1pool.tile([128, 8, DFF // 4], BF16, tag="w1",
                            name=f"w1_{k}_{g}") for g in range(4)]
               for k in range(K)]
        # group 0 streams in s-slices (region-level hazards let the s=0
        # matmuls start after the first 256KB instead of a whole 2MB
        # quarter); later groups as progressively coarser DMAs, all on the
        # sync ring which is the only one pulling at t=0. pbt (needed only
        # by the gating at the end of group 0) queues behind the hot path.
        for lo, n in ((0, 1), (1, 2), (3, 2), (5, 3)):
            nc.sync.dma_start(w1q[0][0][:, ds(lo, n)],
                              w1p[0][0][:, ds(lo, n), :])
            if lo == 0:
                nc.sync.dma_start(xt_sb[:, ds(1, 7)], xTp[:, ds(1, 7), :])
            elif lo == 1:
                nc.sync.dma_start(
                    pbt_sb, pbT.ap().rearrange("(t p) k -> p t k", p=128))
        for h in range(2):
            nc.sync.dma_start(w1q[0][1][:, ds(4 * h, 4)],
                              w1p[0][1][:, ds(4 * h, 4), :])
        for g in range(2, 4):
            nc.sync.dma_start(w1q[0][g], w1p[0][g])

        # gating state: gexp[tok, th, k] = exp(logit), rden[tok, th] = 1/sum
        gexp = small.tile([128, 2, K], F32, tag="gexp")
        rden = small.tile([128, 2], F32, tag="rden")
        den = small.tile([128, 2], F32, tag="den")
        acc = small.tile([128, 2, D], F32, tag="acc")

        ht_t = [None] * K
        w2tls = [[None] * 8 for _ in range(K)]

        def w2_load(k, q, eng):
            w2tls[k][q] = w2pool.tile([128, 4, D], BF16, tag="w2",
                                      name=f"w2_{k}_{q}")
            eng.dma_start(w2tls[k][q], w2p[k][:, ds(4 * q, 4), :])

        def emit_gating():
            # logits^T [tok, K] via x-stationary matmuls (f=4, trivial PE
            # time), softmax along the free axis.
            for th in range(2):
                lg = pp2.tile([128, K], F32, tag="ps2", name=f"lg{th}")
                for s in range(8):
                    nc.tensor.matmul(lg, xt_sb[:, s, ts(th, 128)],
                                     pbt_sb[:, s],
                                     start=(s == 0), stop=(s == 7))
                nc.scalar.activation(gexp[:, th], lg, AF.Exp)
            for th in range(2):
                nc.vector.tensor_reduce(den[:, ds(th, 1)], gexp[:, th],
                                        mybir.AxisListType.X, ALU.add)
                nc.vector.reciprocal(rden[:, ds(th, 1)], den[:, ds(th, 1)])

        def ffn1_half(k, b, half, ps1):
            # 16 matmuls: chains j=0..3 of block b=(g, jh), s-steps
            # 4*half..4*half+3; each chain owns a full PSUM bank.
            g, jh = b // 2, b % 2
            for s in range(4 * half, 4 * half + 4):
                for j in range(4):
                    jj = jh * 4 + j
                    nc.tensor.matmul(ps1[j][:, ds(0, NTC)],
                                     w1q[k][g][:, s, ts(jj, 128)],
                                     xt_sb[:, s],
                                     start=(s == 0), stop=(s == 7))
            if half == 1:
                cols = g * 8 + jh * 4
                for j in range(4):
                    nc.scalar.activation(ht_t[k][:, cols + j],
                                         ps1[j][:, ds(0, NTC)],
                                         AF.Gelu_apprx_tanh)

        def new_ps1(k, b):
            return [pp1.tile([128, 512], F32, tag="ps1",
                             name=f"f1_{k}_{b}_{j}") for j in range(4)]

        def ffn2_step(k, ps2, q):
            for kc in range(2):
                s2 = 2 * q + kc
                for th in range(2):
                    for dc in range(2):
                        nc.tensor.matmul(
                            ps2[th * 2 + dc], ht_t[k][:, s2, ts(th, 128)],
                            w2tls[k][s2 // 4][:, s2 % 4, ts(dc, 512)],
                            start=(s2 == 0), stop=(s2 == 31))

        def drain(k, ps2c, th, dc):
            g1 = gexp[:, th, ds(k, 1)]
            g2 = rden[:, ds(th, 1)]
            c = th * 2 + dc
            if k == 0:
                nc.vector.tensor_scalar(
                    acc[:, th, ts(dc, 512)], ps2c, g1, g2,
                    ALU.mult, ALU.mult)
            elif k < K - 1:
                ob = obp.tile([128, 512], F32, tag="ob", name=f"ob_{k}_{c}")
                nc.vector.tensor_scalar(ob, ps2c, g1, g2, ALU.mult, ALU.mult)
                nc.vector.tensor_add(acc[:, th, ts(dc, 512)],
                                     acc[:, th, ts(dc, 512)], ob)
            else:
                ob = obp.tile([128, 512], F32, tag="ob", name=f"ob_{k}_{c}")
                nc.vector.tensor_scalar(ob, ps2c, g1, g2, ALU.mult, ALU.mult)
                obf = obp.tile([128, 512], BF16, tag="obf", name=f"obf_{c}")
                nc.vector.tensor_add(obf, acc[:, th, ts(dc, 512)], ob)
                nc.sync.dma_start(opiece[:, th, ts(dc, 512)], obf)

        # ---- W_0: path-0 ffn1 standalone + gating ----
        # 4-chain blocks alternating pp1 (combined tile, one 4-wide gelu)
        # and pp2 (singles): consecutive blocks use different pools, so a
        # block never waits on the previous block's gelu.
        ht_t[0] = htp.tile([128, 32, NTC], BF16, tag="ht", name="ht_0")
        # 8-chain groups (no concurrent ffn2, so borrow pp2's 4 banks):
        # gelu of group g overlaps group g+1's chains with no WAR stall.
        # Triggers fire in delivery-deadline order per ring, and w2 tiles
        # allocate in Q-ascending order so pool slots line up with death
        # order (tile (k,Q) always recycles (k-1,Q)'s slot). Sync takes
        # Q0-1 behind the W1 stream; scalar takes w1[1] + Q2-3.
        sched = [[("w2s", 0), ("w1", 0)],
                 [("w2s", 1), ("w2", 2), ("w1", 1)],
                 [("w2", 3), ("w1", 2)],
                 [("w1", 3)]]
        for grp in range(4):
            ps1 = [(pp1 if j < 4 else pp2).tile(
                [128, 512], F32, tag=("ps1" if j < 4 else "ps2"),
                name=f"f1_0_{grp}_{j}") for j in range(8)]
            for s in range(8):
                for j in range(8):
                    nc.tensor.matmul(ps1[j][:, ds(0, NTC)],
                                     w1q[0][grp][:, s, ts(j, 128)],
                                     xt_sb[:, s],
                                     start=(s == 0), stop=(s == 7))
            for j in range(8):
                nc.scalar.activation(ht_t[0][:, grp * 8 + j],
                                     ps1[j][:, ds(0, NTC)],
                                     AF.Gelu_apprx_tanh)
            if grp == 0:
                emit_gating()
            for kind, idx in sched[grp]:
                if kind == "w1":
                    nc.scalar.dma_start(w1q[1][idx], w1p[1][idx])
                elif kind == "w2s":
                    w2_load(0, idx, nc.sync)
                else:
                    w2_load(0, idx, nc.scalar)

        # ---- W_k (k=1..3): ffn2 of k-1 interleaved with ffn1 of k ----
        for k in range(1, K):
            prev = k - 1
            ht_t[k] = htp.tile([128, 32, NTC], BF16, tag="ht", name=f"ht_{k}")
            ps2 = [pp2.tile([128, 512], F32, tag="ps2", name=f"f2_{prev}_{c}")
                   for c in range(4)]
            ps1 = None
            for q in range(16):
                ffn2_step(prev, ps2, q)
                b, half = q // 2, q % 2
                if half == 0:
                    ps1 = new_ps1(k, b)
                ffn1_half(k, b, half, ps1)
                # triggers (block ends / early iters), in ring-FIFO
                # need-order; all WAR-free by the time they're reached.
                # W_1's Q4-7 prefetches ride sync (idle once the W_0 ramp
                # drains) instead of queueing behind 10MB of scalar freight.
                if q < 4:
                    w2_load(prev, q + 4, nc.sync if k == 1 else nc.scalar)
                if half == 1:
                    if b < 4:
                        w2_load(k, b, nc.scalar)
                    elif k == K - 1:
                        # last path: no ffn2-k3 window follows, so pull its
                        # w2 second half in as (2, Q4..7) slots free up
                        w2_load(k, b, nc.scalar)
                    if q % 4 == 3 and k + 1 < K:
                        g = q // 4
                        nc.scalar.dma_start(w1q[k + 1][g], w1p[k + 1][g])
            for th in range(2):
                for dc in range(2):
                    drain(prev, ps2[th * 2 + dc], th, dc)

        # ---- ffn2 of the last path: chain-outer, w2 fully resident ----
        for th in range(2):
            for dc in range(2):
                ps2c = pp2.tile([128, 512], F32, tag="ps2",
                                name=f"f2_3_{th}{dc}")
                for s2 in range(32):
                    nc.tensor.matmul(
                        ps2c, ht_t[3][:, s2, ts(th, 128)],
                        w2tls[3][s2 // 4][:, s2 % 4, ts(dc, 512)],
                        start=(s2 == 0), stop=(s2 == 31))
                drain(3, ps2c, th, dc)


def build(verbose=False):
    nc = bacc.Bacc("TRN2", target_bir_lowering=False, debug=False, num_devices=NCORES)
    with tile.TileContext(nc) as tc:
        _emit(nc, tc)
    nc.compile()
    return nc


def _expand_tt(core1, core2, din, dout):
    a, x, r = core1.shape
    r2, b, y = core2.shape
    m = core1.reshape(a * x, r).astype(np.float32) @ \
        core2.reshape(r2, b * y).astype(np.float32)
    w = m.reshape(a, x, b, y).transpose(0, 2, 1, 3).reshape(a * b, x * y)
    assert w.shape == (din, dout)
    return w


def make_in_maps(inputs):
    tokens = inputs["tokens"]
    bf = ml_dtypes.bfloat16
    shared = {}
    for k in range(K):
        w1 = _expand_tt(inputs["ffn1_core1"][k], inputs["ffn1_core2"][k], D, DFF)
        # [(s p), (g fq)] -> [g, p, s, fq]
        shared[f"w1p{k}"] = np.ascontiguousarray(
            w1.reshape(8, 128, 4, DFF // 4).transpose(2, 1, 0, 3)).astype(bf)
        w2 = _expand_tt(inputs["ffn2_core1"][k], inputs["ffn2_core2"][k], DFF, D)
        w2 *= (1.0 + inputs["path_weights"][k])[None, :]
        shared[f"w2p{k}"] = np.ascontiguousarray(
            w2.reshape(32, 128, D).transpose(1, 0, 2)).astype(bf)
    shared["pbT"] = np.ascontiguousarray(inputs["path_bases"].T).astype(bf)
    in_maps = []
    for c in range(NCORES):
        tok = tokens[c * NTC:(c + 1) * NTC]
        xt = np.ascontiguousarray(
            tok.T.reshape(8, 128, NTC).transpose(1, 0, 2)).astype(bf)
        m = dict(shared)
        m["xTp"] = xt
        in_maps.append(m)
    return in_maps


def assemble(results):
    out = np.empty((NTOK, D), np.float32)
    for c in range(NCORES):
        # piece [128 p, 2 th, 1024 d]; token = c*256 + th*128 + p
        piece = results[c]["opiece"].astype(np.float32)
        out[c * NTC:(c + 1) * NTC] = piece.transpose(1, 0, 2).reshape(NTC, D)
    return out


_NC = None


def run(inputs, trace=False):
    global _NC
    if _NC is None:
        _NC = build()
    res = run_bass_kernel_spmd(
        _NC, make_in_maps(inputs), core_ids=list(range(NCORES)), trace=trace
    )
    return assemble(res.results), res


def kernel(**inputs):
    out, _ = run(inputs)
    return out
